# revision 11
# baseline (speedup 1.0000x reference)
"""Single-head causal cross-attention on 8 Trainium2 NeuronCores.

Problem: B=8, S=2048, D=1024, HS=64 (fp32).
    q = query @ Wq ; k = key @ Wk ; v = value @ Wv        [B, S, HS]
    out = softmax(causal(q k^T / sqrt(HS))) @ v           [B, S, HS]

Sharding: batch across the 8 cores (one batch element per core), weights
replicated. No collectives.

Per-core design (memory regime: ~24 MiB of HBM reads/core is the floor):

* The projections contract over d=1024, so query/key/value must reach the PE
  with d on the partition axis, but they are [s, d] row-major fp32 in HBM and
  DMA-transpose only supports 2-byte dtypes. We view each fp32 element as two
  2-byte units (lo mantissa half / hi half = truncated bf16) and run the
  transposing DMA over fully CONTIGUOUS [512, 2048-unit] blocks; the xbar
  lands unit 128g+p of row s at out[p, g, s]. Odd partitions then carry the
  truncated-bf16 values, even partitions carry garbage.
* Garbage partitions are sanitized with a bitwise-AND against a per-partition
  0x0000/0xFFFF mask on DVE (tensor_scalar, 4x mode, NaN-proof since it is
  not arithmetic), and the matching weight rows are zero, so they contribute
  exactly 0 to the PE contraction.
* Weights are staged through an internal DRAM buffer to interleave zero rows
  (DRAM APs can scatter rows; SBUF partitions cannot), scaled by (1 + 2^-9)
  to compensate the truncation's toward-zero bias, and rounded to bf16.
* Projections run in bf16 on the PE at full rate producing qT/kT/vT [64, S]
  fp32.
* Scores are computed TRANSPOSED (scoresT[k, q] = kT.T @ qT, fp32r at full
  PE rate) so softmax's reduction runs along the PE contraction axis: exp on
  ACT (1/sqrt(HS) scale fused, no max-subtraction needed -- |scores| <~ 8 by
  construction), then one PV accumulation group with v_ext = [v | 1] computes
  both sum_k exp*v and the softmax denominator.
* The small [65, S] result is PE-transposed back, rows normalized by the
  reciprocal of column 64 on DVE, and stored with one batched DMA per chunk.
"""

import sys

for _p in ("/opt/trn_rl_repo",):
    if _p not in sys.path:
        sys.path.insert(0, _p)

import numpy as np

import concourse.bass as bass
import concourse.mybir as mybir
import concourse.tile as tile
from concourse import bacc
from concourse.masks import make_identity

B, S, D, HS = 8, 2048, 1024, 64
N_CORES = 8
QC = 512            # q/s chunk (matmul moving free dim)
KT = 128            # k-tile
NG = D // 64        # 16 unit-groups of 128 units (64 d-values) each
N_QC = S // QC      # 4
N_KT = S // KT      # 16
W_COMP = 1.0 + 2.0 ** -9   # compensates bf16-truncation bias of the inputs

F32 = mybir.dt.float32
F32R = mybir.dt.float32r
BF16 = mybir.dt.bfloat16
U16 = mybir.dt.uint16
I32 = mybir.dt.int32


def build_body(tc, out_d, q_d, k_d, v_d, w_d):
    nc = tc.nc
    Exp = mybir.ActivationFunctionType.Exp
    AND = mybir.AluOpType.bitwise_and

    with tc.tile_pool(name="const", bufs=1) as const_pool:
        # PE transposes run in plain fp32 (fp32r transpose fails ISA
        # codegen); fp32r rounding happens at the ACT copies that produce
        # matmul operands.
        ident = const_pool.tile([128, 128], F32, tag="ident")
        make_identity(nc, ident[:])
        ones_col = const_pool.tile([128, 1], F32, tag="ones_col")
        nc.gpsimd.memset(ones_col[:], 1.0)

        # odd-partition keep-mask for the unit-interleaved layout:
        # fp32 per-partition scalar, 65535.0 on odd partitions / 0.0 on even.
        # Sanitize = min(uint16_view, mask) -- integer-valued comparison, so
        # no NaN can ever be produced regardless of ALU dtype promotion.
        pidx = const_pool.tile([128, 1], I32, tag="pidx")
        nc.gpsimd.iota(pidx[:], pattern=[[1, 1]], base=0, channel_multiplier=1)
        podd = const_pool.tile([128, 1], I32, tag="podd")
        nc.vector.tensor_scalar(podd[:], pidx[:], 1, None, op0=AND)
        pmask_i = const_pool.tile([128, 1], I32, tag="pmask_i")
        nc.vector.tensor_scalar(pmask_i[:], podd[:], 0xFFFF, None,
                                op0=mybir.AluOpType.mult)
        andmask = const_pool.tile([128, 1], F32, tag="andmask")
        nc.vector.tensor_copy(andmask[:], pmask_i[:])

        # Diagonal-block causal masks: mask01[j][k_l, q_l] = 1.0 iff
        # q_l >= k_l + 128*j, else 0.0.
        masks = []
        for j in range(QC // KT):
            m = const_pool.tile([128, QC], F32, tag=f"mask{j}", name=f"mask{j}")
            nc.gpsimd.memset(m[:], 1.0)
            nc.gpsimd.affine_select(
                out=m[:],
                in_=m[:],
                compare_op=mybir.AluOpType.is_ge,
                fill=0.0,
                base=-(KT * j),
                pattern=[[1, QC]],
                channel_multiplier=-1,
            )
            masks.append(m)

        # ---- weights: DRAM-staged interleave (zero even rows / W on odd),
        # scale by W_COMP, round to bf16, AND-sanitize even partitions.
        # All W DMAs ride the ACT HWDGE ring to keep the SP ring free for the
        # big input loads.
        w_all = []
        with (
            tc.tile_pool(name="wstage", bufs=1, space="DRAM") as wstage_pool,
            tc.tile_pool(name="wtmp", bufs=2) as wtmp_pool,
        ):
            for wi in range(3):
                wst = wstage_pool.tile([2 * D, HS], F32, tag=f"wst{wi}",
                                       name=f"wst{wi}")
                # zero everything (even rows stay zero)
                ztmp = wtmp_pool.tile([128, D], F32, tag="ztmp", name="ztmp")
                nc.gpsimd.memset(ztmp[:], 0.0)
                nc.scalar.dma_start(out=wst[:], in_=ztmp[:])
                # odd rows <- W
                odd = wst[:].rearrange("(d two) h -> d two h", two=2)[:, 1, :]
                nc.scalar.dma_start(out=odd, in_=w_d[wi].ap()[:])
                # load as [p, g, h] with row = 128g + p
                wtmp = wtmp_pool.tile([128, NG, HS], F32, tag="wtmp", name="wtmp")
                nc.scalar.dma_start(
                    out=wtmp[:],
                    in_=wst[:].rearrange("(g p) h -> p g h", p=128),
                )
                wa = const_pool.tile([128, NG, HS], BF16, tag=f"w{wi}",
                                     name=f"w{wi}")
                nc.scalar.mul(
                    wa[:].rearrange("p g h -> p (g h)"),
                    wtmp[:].rearrange("p g h -> p (g h)"),
                    W_COMP,
                )
                nc.vector.tensor_scalar(
                    wa[:].rearrange("p g h -> p (g h)").bitcast(U16),
                    wa[:].rearrange("p g h -> p (g h)").bitcast(U16),
                    andmask[:],
                    None,
                    op0=mybir.AluOpType.min,
                )
                w_all.append(wa)

        # ------- inputs: chunked transposed loads + projections -------
        with (
            tc.tile_pool(name="xt", bufs=4) as xt_pool,
            tc.tile_pool(name="projsb", bufs=1) as proj_sb_pool,
            tc.tile_pool(name="vext", bufs=1) as vext_pool,
            tc.tile_pool(name="pacc", bufs=2, space="PSUM") as psum_acc,
        ):
            projT = {}
            for xi, (name, xd) in enumerate([("q", q_d), ("k", k_d), ("v", v_d)]):
                xbf = xd.ap().bitcast(BF16)  # [S, 2D] units
                pT = proj_sb_pool.tile([HS, S], F32 if name == "v" else F32R,
                                       tag=f"{name}T", name=f"{name}T")
                for c in range(N_QC):
                    xt = xt_pool.tile([128, NG, QC], BF16, tag="xt", name="xt")
                    nc.sync.dma_start(
                        out=xt[:],
                        in_=xbf[c * QC:(c + 1) * QC, :],
                        transpose=True,
                    )
                    flat = xt[:].rearrange("p g s -> p (g s)").bitcast(U16)
                    nc.vector.tensor_scalar(flat, flat, andmask[:], None,
                                            op0=mybir.AluOpType.min)
                    acc = psum_acc.tile([HS, QC], F32, tag="acc", name="acc")
                    for g in range(NG):
                        nc.tensor.matmul(
                            acc[:],
                            lhsT=w_all[xi][:, g, :],
                            rhs=xt[:, g, :],
                            start=(g == 0),
                            stop=(g == NG - 1),
                        )
                    nc.scalar.copy(pT[:, c * QC:(c + 1) * QC], acc[:])
                projT[name] = pT

            # v_ext[kt] = [v_rows | 1] : [128, HS+1] per k-tile.
            v_ext = []
            for kt in range(N_KT):
                pt = psum_acc.tile([KT, HS], F32, tag="acc", name="vtr")
                nc.tensor.transpose(
                    pt[:, 0:HS],
                    projT["v"][:, kt * KT:(kt + 1) * KT],
                    ident[0:HS, 0:HS],
                )
                vx = vext_pool.tile([KT, HS + 1], F32R, tag=f"vext{kt}",
                                    name=f"vext{kt}")
                nc.scalar.copy(vx[:, 0:HS], pt[:, 0:HS])
                nc.scalar.copy(vx[:, HS:HS + 1], ones_col[:])
                v_ext.append(vx)

            # ------------------- attention -------------------
            qT, kTt = projT["q"], projT["k"]
            with (
                tc.tile_pool(name="pscore", bufs=2, space="PSUM") as psum_s,
                tc.tile_pool(name="pu", bufs=1, space="PSUM") as psum_u,
                tc.tile_pool(name="expp", bufs=4) as exp_pool,
                tc.tile_pool(name="usb", bufs=2) as usb_pool,
                tc.tile_pool(name="outsb", bufs=2) as out_pool,
                tc.tile_pool(name="recip", bufs=4) as recip_pool,
            ):
                for qc in range(N_QC):
                    u = psum_u.tile([HS + 1, QC], F32, tag=f"u{qc}",
                                    name=f"u{qc}")
                    n_kt = (qc + 1) * (QC // KT)
                    for kt in range(n_kt):
                        st = psum_s.tile([KT, QC], F32, tag="st", name="st")
                        nc.tensor.matmul(
                            st[:],
                            lhsT=kTt[:, kt * KT:(kt + 1) * KT],
                            rhs=qT[:, qc * QC:(qc + 1) * QC],
                        )
                        et = exp_pool.tile([KT, QC], F32R, tag="et", name="et")
                        nc.scalar.activation(et[:], st[:], Exp,
                                             scale=float(HS) ** -0.5)
                        j = kt - qc * (QC // KT)
                        if j >= 0:  # diagonal block: zero the invalid region
                            nc.vector.tensor_mul(et[:], et[:], masks[j][:])
                        nc.tensor.matmul(
                            u[:],
                            lhsT=v_ext[kt][:],
                            rhs=et[:],
                            start=(kt == 0),
                            stop=(kt == n_kt - 1),
                        )
                    # transpose back, normalize, store
                    usb = usb_pool.tile([HS + 1, QC], F32, tag="usb", name="usb")
                    nc.scalar.copy(usb[:], u[:])
                    osb = out_pool.tile([128, (QC // 128) * HS], F32,
                                        tag="osb", name="osb")
                    for t in range(QC // 128):
                        po = psum_s.tile([128, QC], F32, tag="st", name="po")
                        nc.tensor.transpose(
                            po[:, 0:HS + 1],
                            usb[:, t * 128:(t + 1) * 128],
                            ident[0:HS + 1, 0:HS + 1],
                        )
                        rc = recip_pool.tile([128, 1], F32, tag="rc", name="rc")
                        nc.vector.reciprocal(rc[:], po[:, HS:HS + 1])
                        nc.vector.tensor_scalar_mul(
                            osb[:, t * HS:(t + 1) * HS], po[:, 0:HS], rc[:]
                        )
                    dst = (
                        out_d.ap()[qc * QC:(qc + 1) * QC, :]
                        .rearrange("(t p) h -> p t h", p=128)
                    )
                    nc.sync.dma_start(
                        out=dst,
                        in_=osb[:].rearrange("p (t h) -> p t h", t=QC // 128),
                    )


_NC_CACHE = {}


def build_nc(debug=False):
    key = ("nc", debug)
    if key in _NC_CACHE:
        return _NC_CACHE[key]
    nc = bacc.Bacc(
        "TRN2",
        target_bir_lowering=False,
        debug=debug,
        num_devices=N_CORES,
    )
    q_d = nc.dram_tensor("query", [S, D], F32, kind="ExternalInput")
    k_d = nc.dram_tensor("key", [S, D], F32, kind="ExternalInput")
    v_d = nc.dram_tensor("value", [S, D], F32, kind="ExternalInput")
    wq_d = nc.dram_tensor("Wq", [D, HS], F32, kind="ExternalInput")
    wk_d = nc.dram_tensor("Wk", [D, HS], F32, kind="ExternalInput")
    wv_d = nc.dram_tensor("Wv", [D, HS], F32, kind="ExternalInput")
    out_d = nc.dram_tensor("out", [S, HS], F32, kind="ExternalOutput")

    with tile.TileContext(nc) as tc:
        build_body(tc, out_d, q_d, k_d, v_d, [wq_d, wk_d, wv_d])
    nc.compile()
    _NC_CACHE[key] = nc
    return nc


def make_in_maps(query, key, value, Wq, Wk, Wv):
    query = np.ascontiguousarray(query, dtype=np.float32)
    key = np.ascontiguousarray(key, dtype=np.float32)
    value = np.ascontiguousarray(value, dtype=np.float32)
    Wq = np.ascontiguousarray(Wq, dtype=np.float32)
    Wk = np.ascontiguousarray(Wk, dtype=np.float32)
    Wv = np.ascontiguousarray(Wv, dtype=np.float32)
    return [
        {
            "query": query[b],
            "key": key[b],
            "value": value[b],
            "Wq": Wq,
            "Wk": Wk,
            "Wv": Wv,
        }
        for b in range(N_CORES)
    ]


def kernel(query, key, value, Wq, Wk, Wv, trace=False):
    from concourse.bass_utils import run_bass_kernel_spmd

    nc = build_nc()
    in_maps = make_in_maps(query, key, value, Wq, Wk, Wv)
    res = run_bass_kernel_spmd(nc, in_maps, core_ids=list(range(N_CORES)), trace=trace)
    out = np.stack([res.results[b]["out"] for b in range(N_CORES)], axis=0)
    if trace:
        kernel.last_results = res
    return out


# revision 12
# speedup vs baseline: 12.1685x; 12.1685x over previous
"""Single-head causal cross-attention on 8 Trainium2 NeuronCores.

Problem: B=8, S=2048, D=1024, HS=64 (fp32).
    q = query @ Wq ; k = key @ Wk ; v = value @ Wv        [B, S, HS]
    out = softmax(causal(q k^T / sqrt(HS))) @ v           [B, S, HS]

Sharding: batch across the 8 cores (one batch element per core), weights
replicated. No collectives.

Per-core design (memory regime: ~24 MiB of HBM reads/core is the floor):

* The projections contract over d=1024, so query/key/value must reach the PE
  with d on the partition axis, but they are [s, d] row-major fp32 in HBM and
  DMA-transpose only supports 2-byte dtypes. We view each fp32 element as two
  2-byte units (lo mantissa half / hi half = truncated bf16) and run the
  transposing DMA over fully CONTIGUOUS [512, 2048-unit] blocks; the xbar
  lands unit 128g+p of row s at out[p, g, s]. Odd partitions then carry the
  truncated-bf16 values, even partitions carry garbage.
* Garbage partitions are sanitized with a bitwise-AND against a per-partition
  0x0000/0xFFFF mask on DVE (tensor_scalar, 4x mode, NaN-proof since it is
  not arithmetic), and the matching weight rows are zero, so they contribute
  exactly 0 to the PE contraction.
* Weights are staged through an internal DRAM buffer to interleave zero rows
  (DRAM APs can scatter rows; SBUF partitions cannot), scaled by (1 + 2^-9)
  to compensate the truncation's toward-zero bias, and rounded to bf16.
* Projections run in bf16 on the PE at full rate producing qT/kT/vT [64, S]
  fp32.
* Scores are computed TRANSPOSED (scoresT[k, q] = kT.T @ qT, fp32r at full
  PE rate) so softmax's reduction runs along the PE contraction axis: exp on
  ACT (1/sqrt(HS) scale fused, no max-subtraction needed -- |scores| <~ 8 by
  construction), then one PV accumulation group with v_ext = [v | 1] computes
  both sum_k exp*v and the softmax denominator.
* The small [65, S] result is PE-transposed back, rows normalized by the
  reciprocal of column 64 on DVE, and stored with one batched DMA per chunk.
"""

import sys

for _p in ("/opt/trn_rl_repo",):
    if _p not in sys.path:
        sys.path.insert(0, _p)

import numpy as np

import concourse.bass as bass
import concourse.mybir as mybir
import concourse.tile as tile
from concourse import bacc
from concourse.masks import make_identity

B, S, D, HS = 8, 2048, 1024, 64
N_CORES = 8
QC = 512            # q/s chunk (matmul moving free dim)
KT = 128            # k-tile
NG = D // 64        # 16 unit-groups of 128 units (64 d-values) each
N_QC = S // QC      # 4
N_KT = S // KT      # 16
W_COMP = 1.0 + 2.0 ** -9   # compensates bf16-truncation bias of the inputs

F32 = mybir.dt.float32
F32R = mybir.dt.float32r
BF16 = mybir.dt.bfloat16
U16 = mybir.dt.uint16
I32 = mybir.dt.int32


def build_body(tc, out_d, q_d, k_d, v_d, w_d):
    nc = tc.nc
    Exp = mybir.ActivationFunctionType.Exp
    AND = mybir.AluOpType.bitwise_and

    with tc.tile_pool(name="const", bufs=1) as const_pool:
        # PE transposes run in plain fp32 (fp32r transpose fails ISA
        # codegen); fp32r rounding happens at the ACT copies that produce
        # matmul operands.
        ident = const_pool.tile([128, 128], F32, tag="ident")
        make_identity(nc, ident[:])
        ones_col = const_pool.tile([128, 1], F32, tag="ones_col")
        nc.gpsimd.memset(ones_col[:], 1.0)

        # odd-partition keep-mask for the unit-interleaved layout:
        # fp32 per-partition scalar, 65535.0 on odd partitions / 0.0 on even.
        # Sanitize = min(uint16_view, mask) -- integer-valued comparison, so
        # no NaN can ever be produced regardless of ALU dtype promotion.
        pidx = const_pool.tile([128, 1], I32, tag="pidx")
        nc.gpsimd.iota(pidx[:], pattern=[[1, 1]], base=0, channel_multiplier=1)
        podd = const_pool.tile([128, 1], I32, tag="podd")
        nc.vector.tensor_scalar(podd[:], pidx[:], 1, None, op0=AND)
        pmask_i = const_pool.tile([128, 1], I32, tag="pmask_i")
        nc.vector.tensor_scalar(pmask_i[:], podd[:], 0xFFFF, None,
                                op0=mybir.AluOpType.mult)
        andmask = const_pool.tile([128, 1], F32, tag="andmask")
        nc.vector.tensor_copy(andmask[:], pmask_i[:])

        # Diagonal-block causal masks: mask01[j][k_l, q_l] = 1.0 iff
        # q_l >= k_l + 128*j, else 0.0.
        masks = []
        for j in range(QC // KT):
            m = const_pool.tile([128, QC], F32, tag=f"mask{j}", name=f"mask{j}")
            nc.gpsimd.memset(m[:], 1.0)
            nc.gpsimd.affine_select(
                out=m[:],
                in_=m[:],
                compare_op=mybir.AluOpType.is_ge,
                fill=0.0,
                base=-(KT * j),
                pattern=[[1, QC]],
                channel_multiplier=-1,
            )
            masks.append(m)

        # ---- weights: DRAM-staged interleave (zero even rows / W on odd),
        # scale by W_COMP, round to bf16, AND-sanitize even partitions.
        # All W DMAs ride the ACT HWDGE ring to keep the SP ring free for the
        # big input loads.
        w_all = []
        with (
            tc.tile_pool(name="wstage", bufs=1, space="DRAM") as wstage_pool,
            tc.tile_pool(name="wtmp", bufs=2) as wtmp_pool,
        ):
            for wi in range(3):
                wst = wstage_pool.tile([2 * D, HS], F32, tag=f"wst{wi}",
                                       name=f"wst{wi}")
                # zero everything (even rows stay zero)
                ztmp = wtmp_pool.tile([128, D], F32, tag="ztmp", name="ztmp")
                nc.gpsimd.memset(ztmp[:], 0.0)
                nc.scalar.dma_start(out=wst[:], in_=ztmp[:])
                # odd rows <- W
                odd = wst[:].rearrange("(d two) h -> d two h", two=2)[:, 1, :]
                nc.scalar.dma_start(out=odd, in_=w_d[wi].ap()[:])
                # load as [p, g, h] with row = 128g + p
                wtmp = wtmp_pool.tile([128, NG, HS], F32, tag="wtmp", name="wtmp")
                nc.scalar.dma_start(
                    out=wtmp[:],
                    in_=wst[:].rearrange("(g p) h -> p g h", p=128),
                )
                wa = const_pool.tile([128, NG, HS], BF16, tag=f"w{wi}",
                                     name=f"w{wi}")
                nc.scalar.mul(
                    wa[:].rearrange("p g h -> p (g h)"),
                    wtmp[:].rearrange("p g h -> p (g h)"),
                    W_COMP,
                )
                nc.vector.tensor_scalar(
                    wa[:].rearrange("p g h -> p (g h)").bitcast(U16),
                    wa[:].rearrange("p g h -> p (g h)").bitcast(U16),
                    andmask[:],
                    None,
                    op0=mybir.AluOpType.min,
                )
                w_all.append(wa)

        # ------- inputs: chunked transposed loads + projections -------
        with (
            tc.tile_pool(name="xt", bufs=4) as xt_pool,
            tc.tile_pool(name="projsb", bufs=1) as proj_sb_pool,
            tc.tile_pool(name="vext", bufs=1) as vext_pool,
            tc.tile_pool(name="pacc", bufs=2, space="PSUM") as psum_acc,
        ):
            projT = {}
            for xi, (name, xd) in enumerate([("q", q_d), ("k", k_d), ("v", v_d)]):
                xbf = xd.ap().bitcast(BF16)  # [S, 2D] units
                pT = proj_sb_pool.tile([HS, S], F32 if name == "v" else F32R,
                                       tag=f"{name}T", name=f"{name}T")
                for c in range(N_QC):
                    xt = xt_pool.tile([128, NG, QC], BF16, tag="xt", name="xt")
                    nc.sync.dma_start(
                        out=xt[:],
                        in_=xbf[c * QC:(c + 1) * QC, :],
                        transpose=True,
                    )
                    flat = xt[:].rearrange("p g s -> p (g s)").bitcast(U16)
                    nc.vector.tensor_scalar(flat, flat, andmask[:], None,
                                            op0=mybir.AluOpType.min)
                    acc = psum_acc.tile([HS, QC], F32, tag="acc", name="acc")
                    for g in range(NG):
                        nc.tensor.matmul(
                            acc[:],
                            lhsT=w_all[xi][:, g, :],
                            rhs=xt[:, g, :],
                            start=(g == 0),
                            stop=(g == NG - 1),
                        )
                    nc.scalar.copy(pT[:, c * QC:(c + 1) * QC], acc[:])
                projT[name] = pT

            # v_ext[kt] = [v_rows | 1] : [128, HS+1] per k-tile.
            v_ext = []
            for kt in range(N_KT):
                pt = psum_acc.tile([KT, HS], F32, tag="acc", name="vtr")
                nc.tensor.transpose(
                    pt[:, 0:HS],
                    projT["v"][:, kt * KT:(kt + 1) * KT],
                    ident[0:HS, 0:HS],
                )
                vx = vext_pool.tile([KT, HS + 1], F32R, tag=f"vext{kt}",
                                    name=f"vext{kt}")
                nc.scalar.copy(vx[:, 0:HS], pt[:, 0:HS])
                nc.scalar.copy(vx[:, HS:HS + 1], ones_col[:])
                v_ext.append(vx)

            # ------------------- attention -------------------
            qT, kTt = projT["q"], projT["k"]
            with (
                tc.tile_pool(name="pscore", bufs=2, space="PSUM") as psum_s,
                tc.tile_pool(name="pu", bufs=1, space="PSUM") as psum_u,
                tc.tile_pool(name="expp", bufs=4) as exp_pool,
                tc.tile_pool(name="usb", bufs=2) as usb_pool,
                tc.tile_pool(name="outsb", bufs=2) as out_pool,
                tc.tile_pool(name="recip", bufs=4) as recip_pool,
            ):
                for qc in range(N_QC):
                    u = psum_u.tile([HS + 1, QC], F32, tag=f"u{qc}",
                                    name=f"u{qc}")
                    n_kt = (qc + 1) * (QC // KT)
                    for kt in range(n_kt):
                        st = psum_s.tile([KT, QC], F32, tag="st", name="st")
                        nc.tensor.matmul(
                            st[:],
                            lhsT=kTt[:, kt * KT:(kt + 1) * KT],
                            rhs=qT[:, qc * QC:(qc + 1) * QC],
                        )
                        et = exp_pool.tile([KT, QC], F32R, tag="et", name="et")
                        nc.scalar.activation(et[:], st[:], Exp,
                                             scale=float(HS) ** -0.5)
                        j = kt - qc * (QC // KT)
                        if j >= 0:  # diagonal block: zero the invalid region
                            nc.vector.tensor_mul(et[:], et[:], masks[j][:])
                        nc.tensor.matmul(
                            u[:],
                            lhsT=v_ext[kt][:],
                            rhs=et[:],
                            start=(kt == 0),
                            stop=(kt == n_kt - 1),
                        )
                    # transpose back, normalize, store
                    usb = usb_pool.tile([HS + 1, QC], F32, tag="usb", name="usb")
                    nc.scalar.copy(usb[:], u[:])
                    osb = out_pool.tile([128, (QC // 128) * HS], F32,
                                        tag="osb", name="osb")
                    for t in range(QC // 128):
                        po = psum_s.tile([128, QC], F32, tag="st", name="po")
                        nc.tensor.transpose(
                            po[:, 0:HS + 1],
                            usb[:, t * 128:(t + 1) * 128],
                            ident[0:HS + 1, 0:HS + 1],
                        )
                        rc = recip_pool.tile([128, 1], F32, tag="rc", name="rc")
                        nc.vector.reciprocal(rc[:], po[:, HS:HS + 1])
                        nc.vector.tensor_scalar_mul(
                            osb[:, t * HS:(t + 1) * HS], po[:, 0:HS], rc[:]
                        )
                    dst = (
                        out_d.ap()[qc * QC:(qc + 1) * QC, :]
                        .rearrange("(t p) h -> p t h", p=128)
                    )
                    nc.sync.dma_start(
                        out=dst,
                        in_=osb[:].rearrange("p (t h) -> p t h", t=QC // 128),
                    )


_NC_CACHE = {}


def build_nc(debug=False, reps=1):
    key = ("nc", debug, reps)
    if key in _NC_CACHE:
        return _NC_CACHE[key]
    nc = bacc.Bacc(
        "TRN2",
        target_bir_lowering=False,
        debug=debug,
        num_devices=N_CORES,
    )
    q_d = nc.dram_tensor("query", [S, D], F32, kind="ExternalInput")
    k_d = nc.dram_tensor("key", [S, D], F32, kind="ExternalInput")
    v_d = nc.dram_tensor("value", [S, D], F32, kind="ExternalInput")
    wq_d = nc.dram_tensor("Wq", [D, HS], F32, kind="ExternalInput")
    wk_d = nc.dram_tensor("Wk", [D, HS], F32, kind="ExternalInput")
    wv_d = nc.dram_tensor("Wv", [D, HS], F32, kind="ExternalInput")
    out_d = nc.dram_tensor("out", [S, HS], F32, kind="ExternalOutput")

    with tile.TileContext(nc) as tc:
        for _ in range(reps):
            build_body(tc, out_d, q_d, k_d, v_d, [wq_d, wk_d, wv_d])
    nc.compile()
    _NC_CACHE[key] = nc
    return nc


def make_in_maps(query, key, value, Wq, Wk, Wv):
    query = np.ascontiguousarray(query, dtype=np.float32)
    key = np.ascontiguousarray(key, dtype=np.float32)
    value = np.ascontiguousarray(value, dtype=np.float32)
    Wq = np.ascontiguousarray(Wq, dtype=np.float32)
    Wk = np.ascontiguousarray(Wk, dtype=np.float32)
    Wv = np.ascontiguousarray(Wv, dtype=np.float32)
    return [
        {
            "query": query[b],
            "key": key[b],
            "value": value[b],
            "Wq": Wq,
            "Wk": Wk,
            "Wv": Wv,
        }
        for b in range(N_CORES)
    ]


def kernel(query, key, value, Wq, Wk, Wv, trace=False):
    from concourse.bass_utils import run_bass_kernel_spmd

    nc = build_nc()
    in_maps = make_in_maps(query, key, value, Wq, Wk, Wv)
    res = run_bass_kernel_spmd(nc, in_maps, core_ids=list(range(N_CORES)), trace=trace)
    out = np.stack([res.results[b]["out"] for b in range(N_CORES)], axis=0)
    if trace:
        kernel.last_results = res
    return out
